# revision 4
# baseline (speedup 1.0000x reference)
"""Trainium2 Bass kernel for nn_AqtDotGeneral_19481971655318.

Computes the AQT-style int8 quantized matmul:
    lhs_scale = absmax(lhs, axis=K) / 127        # [B,S,1] per row
    rhs_scale = absmax(rhs, axis=K) / 127        # [1,N]   per column
    q_lhs = round(lhs / lhs_scale)  (int8 range)
    q_rhs = round(rhs / rhs_scale)
    out = (q_lhs @ q_rhs) * lhs_scale * rhs_scale

Sharding: data-parallel over B*S rows across 8 cores (4096 rows each);
rhs replicated. Per-core dataflow:
  - lhs row-tile [128,1024] f32 DMA in; DVE absmax-reduce per row
  - quantize: DVE (x*rs + M), GPSIMD (y - M) -> integer-valued bf16
    (M = 1.5*2^23 magic number gives round-half-even, matching jnp.round)
  - xbar DMA transpose -> lhsT [K,M] bf16 tiles
  - TensorE bf16 matmuls accumulate in PSUM over K (exact: integer
    products up to 127*127 are exact in bf16*bf16->f32-accum)
  - per-column rhs scale is folded into q_rhs (bf16), so dequant is one
    ScalarE pass: out = psum * row_scale, PSUM->SBUF, then DMA out.
"""

import numpy as np

N_CORES = 8
B, S, K, N = 4, 8192, 1024, 1024
ROWS_TOTAL = B * S
ROWS_PER_CORE = ROWS_TOTAL // N_CORES  # 4096
P = 128
ROW_TILES = ROWS_PER_CORE // P  # 32
KT = K // P  # 8 k-tiles
NB = N // P  # 8 n-blocks
MAGIC = 12582912.0  # 1.5 * 2**23: float32 round-to-nearest-even trick
INV127 = 1.0 / 127.0
CHUNK = 8  # row-tiles per scale-math batch

_CACHE = {}


def _build():
    import concourse.mybir as mybir
    import concourse.tile as tile
    from concourse import bacc
    from concourse.masks import make_identity

    f32 = mybir.dt.float32
    bf16 = mybir.dt.bfloat16

    nc = bacc.Bacc(None, target_bir_lowering=False, debug=False)
    lhs_d = nc.declare_dram_parameter("lhs", [ROWS_PER_CORE, K], f32, isOutput=False)
    rhs_d = nc.declare_dram_parameter("rhs", [K, N], f32, isOutput=False)
    out_d = nc.declare_dram_parameter("out", [ROWS_PER_CORE, N], f32, isOutput=True)

    with tile.TileContext(nc) as tc:
        with (
            tc.tile_pool(name="qrhs", bufs=1) as qrhs_pool,
            tc.tile_pool(name="lhsx", bufs=2) as lhsx_pool,
            tc.tile_pool(name="scales", bufs=2) as scales_pool,
            tc.tile_pool(name="ybuf", bufs=3) as y_pool,
            tc.tile_pool(name="qbuf", bufs=3) as q_pool,
            tc.tile_pool(name="qlt", bufs=3) as qlt_pool,
            tc.tile_pool(name="obuf", bufs=3) as o_pool,
            tc.tile_pool(name="mm_psum", bufs=2, space="PSUM") as mm_psum,
        ):
            # ---------------- rhs prep (replicated on all cores) ----------
            q_rhs = qrhs_pool.tile([P, KT, N], bf16, name="q_rhs")
            with (
                tc.tile_pool(name="rhsbuf", bufs=1) as rhsbuf_pool,
                tc.tile_pool(name="rhs_misc", bufs=1) as rhs_misc,
                tc.tile_pool(name="rhs_tp_psum", bufs=4, space="PSUM") as rhs_tp_psum,
                tc.tile_pool(name="rhs_q", bufs=2) as rhs_q_pool,
            ):
                ident = rhs_misc.tile([P, P], f32, name="ident")
                make_identity(nc, ident)

                # transpose rhs via TensorE into rhsT [n_part, nb, k],
                # staging one k-tile of rhs at a time
                rhsT = rhsbuf_pool.tile([P, NB, K], f32, name="rhsT")
                for kt in range(KT):
                    rhs_f = rhsbuf_pool.tile(
                        [P, N], f32, name="rhs_f", tag="rhs_f", bufs=2)
                    nc.sync.dma_start(
                        out=rhs_f, in_=rhs_d[kt * P:(kt + 1) * P, :])
                    for nb in range(NB):
                        psT = rhs_tp_psum.tile([P, P], f32, name="psT", tag="psT")
                        nc.tensor.transpose(
                            psT, rhs_f[:, nb * P:(nb + 1) * P], ident)
                        nc.any.tensor_copy(
                            out=rhsT[:, nb, kt * P:(kt + 1) * P], in_=psT)

                # per-column absmax over K (exact, f32), scales
                namax = rhs_misc.tile([P, NB], f32, name="namax")
                for nb in range(NB):
                    nc.vector.tensor_reduce(
                        out=namax[:, nb:nb + 1], in_=rhsT[:, nb, :],
                        axis=mybir.AxisListType.X, op=mybir.AluOpType.max,
                        apply_absolute_value=True,
                    )
                s_col = rhs_misc.tile([P, NB], f32, name="s_col")
                nc.vector.tensor_scalar_mul(s_col, namax, INV127)
                rs_col = rhs_misc.tile([P, NB], f32, name="rs_col")
                nc.vector.reciprocal(rs_col, s_col)

                # quantize+fold col-scale in transposed layout, xbar back
                for nb in range(NB):
                    yT = rhs_q_pool.tile([P, K], f32, name="yT", tag="yT")
                    nc.vector.tensor_scalar(
                        out=yT, in0=rhsT[:, nb, :],
                        scalar1=rs_col[:, nb:nb + 1], scalar2=MAGIC,
                        op0=mybir.AluOpType.mult, op1=mybir.AluOpType.add,
                    )
                    qsT = rhs_q_pool.tile([P, K], bf16, name="qsT", tag="qsT")
                    nc.vector.tensor_scalar(
                        out=qsT, in0=yT,
                        scalar1=MAGIC, scalar2=s_col[:, nb:nb + 1],
                        op0=mybir.AluOpType.subtract, op1=mybir.AluOpType.mult,
                    )
                    qtmp = rhs_q_pool.tile(
                        [P, KT, P], bf16, name=f"qtmp{nb}", tag="qtmp", bufs=NB)
                    nc.scalar.dma_start(out=qtmp, in_=qsT, transpose=True)
                    # repack into q_rhs [k_part, kt, n] (moving operand layout)
                    nc.vector.tensor_copy(
                        out=q_rhs[:, :, nb * P:(nb + 1) * P], in_=qtmp)

            # ---------------- lhs main loop ------------------------------
            for g in range(ROW_TILES // CHUNK):
                xs = []
                amax = scales_pool.tile([P, CHUNK], f32, name="amax", tag="amax")
                for j in range(CHUNK):
                    i = g * CHUNK + j
                    x = lhsx_pool.tile([P, K], f32, name=f"x{j}", tag=f"x{j}")
                    nc.sync.dma_start(out=x, in_=lhs_d[i * P:(i + 1) * P, :])
                    nc.vector.tensor_reduce(
                        out=amax[:, j:j + 1], in_=x,
                        axis=mybir.AxisListType.X, op=mybir.AluOpType.max,
                        apply_absolute_value=True,
                    )
                    xs.append(x)
                s_row = scales_pool.tile([P, CHUNK], f32, name="s_row", tag="s_row")
                nc.vector.tensor_scalar_mul(s_row, amax, INV127)
                rs_row = scales_pool.tile([P, CHUNK], f32, name="rs_row", tag="rs_row")
                nc.vector.reciprocal(rs_row, s_row)

                for j in range(CHUNK):
                    i = g * CHUNK + j
                    x = xs[j]
                    y = y_pool.tile([P, K], f32, name="y", tag="y")
                    nc.vector.tensor_scalar(
                        out=y, in0=x,
                        scalar1=rs_row[:, j:j + 1], scalar2=MAGIC,
                        op0=mybir.AluOpType.mult, op1=mybir.AluOpType.add,
                    )
                    q = q_pool.tile([P, K], bf16, name="q", tag="q")
                    nc.gpsimd.tensor_scalar(
                        out=q, in0=y, scalar1=MAGIC, scalar2=None,
                        op0=mybir.AluOpType.subtract,
                    )
                    qlT = qlt_pool.tile([P, KT, P], bf16, name="qlT", tag="qlT")
                    nc.scalar.dma_start(out=qlT, in_=q, transpose=True)

                    ps0 = mm_psum.tile([P, 512], f32, name="ps0", tag="ps0")
                    ps1 = mm_psum.tile([P, 512], f32, name="ps1", tag="ps1")
                    for b in range(KT):
                        nc.tensor.matmul(
                            ps0, qlT[:, b, :], q_rhs[:, b, 0:512],
                            start=(b == 0), stop=(b == KT - 1),
                        )
                    for b in range(KT):
                        nc.tensor.matmul(
                            ps1, qlT[:, b, :], q_rhs[:, b, 512:1024],
                            start=(b == 0), stop=(b == KT - 1),
                        )
                    o = o_pool.tile([P, N], f32, name="o", tag="o")
                    nc.scalar.activation(
                        out=o[:, 0:512], in_=ps0,
                        func=mybir.ActivationFunctionType.Copy,
                        bias=0.0, scale=s_row[:, j:j + 1],
                    )
                    nc.scalar.activation(
                        out=o[:, 512:1024], in_=ps1,
                        func=mybir.ActivationFunctionType.Copy,
                        bias=0.0, scale=s_row[:, j:j + 1],
                    )
                    nc.sync.dma_start(out=out_d[i * P:(i + 1) * P, :], in_=o)

    nc.compile()
    return nc


def _get_nc():
    if "nc" not in _CACHE:
        _CACHE["nc"] = _build()
    return _CACHE["nc"]


def kernel(lhs: np.ndarray, rhs: np.ndarray) -> np.ndarray:
    from concourse.bass_utils import run_bass_kernel_spmd

    nc = _get_nc()
    lhs_flat = np.ascontiguousarray(lhs.reshape(ROWS_TOTAL, K), dtype=np.float32)
    rhs = np.ascontiguousarray(rhs, dtype=np.float32)
    in_maps = [
        {
            "lhs": lhs_flat[c * ROWS_PER_CORE:(c + 1) * ROWS_PER_CORE],
            "rhs": rhs,
        }
        for c in range(N_CORES)
    ]
    res = run_bass_kernel_spmd(nc, in_maps, core_ids=list(range(N_CORES)))
    out = np.concatenate([res.results[c]["out"] for c in range(N_CORES)], axis=0)
    return out.reshape(B, S, N)


# revision 5
# speedup vs baseline: 1.9761x; 1.9761x over previous
"""Trainium2 Bass kernel for nn_AqtDotGeneral_19481971655318.

Computes the AQT-style int8 quantized matmul:
    lhs_scale = absmax(lhs, axis=K) / 127        # [B,S,1] per row
    rhs_scale = absmax(rhs, axis=K) / 127        # [1,N]   per column
    q_lhs = round(lhs / lhs_scale)  (int8 range)
    q_rhs = round(rhs / rhs_scale)
    out = (q_lhs @ q_rhs) * lhs_scale * rhs_scale

Sharding: data-parallel over B*S rows across 8 cores (4096 rows each);
rhs replicated. Per-core dataflow:
  - lhs row-tile [128,1024] f32 DMA in; DVE absmax-reduce per row
  - quantize: DVE (x*rs + M), GPSIMD (y - M) -> integer-valued bf16
    (M = 1.5*2^23 magic number gives round-half-even, matching jnp.round)
  - xbar DMA transpose -> lhsT [K,M] bf16 tiles
  - TensorE bf16 matmuls accumulate in PSUM over K (exact: integer
    products up to 127*127 are exact in bf16*bf16->f32-accum)
  - per-column rhs scale is folded into q_rhs (bf16), so dequant is one
    ScalarE pass: out = psum * row_scale, PSUM->SBUF, then DMA out.
"""

import numpy as np

N_CORES = 8
B, S, K, N = 4, 8192, 1024, 1024
ROWS_TOTAL = B * S
ROWS_PER_CORE = ROWS_TOTAL // N_CORES  # 4096
P = 128
ROW_TILES = ROWS_PER_CORE // P  # 32
KT = K // P  # 8 k-tiles
NB = N // P  # 8 n-blocks
MAGIC = 12582912.0  # 1.5 * 2**23: float32 round-to-nearest-even trick
INV127 = 1.0 / 127.0
CHUNK = 8  # row-tiles per scale-math batch

_CACHE = {}


def _build():
    import concourse.mybir as mybir
    import concourse.tile as tile
    from concourse import bacc
    from concourse.masks import make_identity

    f32 = mybir.dt.float32
    bf16 = mybir.dt.bfloat16

    nc = bacc.Bacc(None, target_bir_lowering=False, debug=False)
    lhs_d = nc.declare_dram_parameter("lhs", [ROWS_PER_CORE, K], f32, isOutput=False)
    rhs_d = nc.declare_dram_parameter("rhs", [K, N], f32, isOutput=False)
    out_d = nc.declare_dram_parameter("out", [ROWS_PER_CORE, N], f32, isOutput=True)

    with tile.TileContext(nc) as tc:
        with (
            tc.tile_pool(name="qrhs", bufs=1) as qrhs_pool,
            tc.tile_pool(name="lhsx", bufs=2) as lhsx_pool,
            tc.tile_pool(name="scales", bufs=2) as scales_pool,
            tc.tile_pool(name="ybuf", bufs=3) as y_pool,
            tc.tile_pool(name="qbuf", bufs=3) as q_pool,
            tc.tile_pool(name="qlt", bufs=3) as qlt_pool,
            tc.tile_pool(name="obuf", bufs=3) as o_pool,
            tc.tile_pool(name="mm_psum", bufs=2, space="PSUM") as mm_psum,
        ):
            # ---------------- rhs prep (replicated on all cores) ----------
            q_rhs = qrhs_pool.tile([P, KT, N], bf16, name="q_rhs")
            with (
                tc.tile_pool(name="rhsbuf", bufs=1) as rhsbuf_pool,
                tc.tile_pool(name="rhs_misc", bufs=1) as rhs_misc,
                tc.tile_pool(name="rhs_tp_psum", bufs=4, space="PSUM") as rhs_tp_psum,
                tc.tile_pool(name="rhs_q", bufs=2) as rhs_q_pool,
            ):
                ident = rhs_misc.tile([P, P], f32, name="ident")
                make_identity(nc, ident)

                # transpose rhs via TensorE into rhsT [n_part, nb, k],
                # staging one k-tile of rhs at a time
                rhsT = rhsbuf_pool.tile([P, NB, K], f32, name="rhsT")
                for kt in range(KT):
                    rhs_f = rhsbuf_pool.tile(
                        [P, N], f32, name="rhs_f", tag="rhs_f", bufs=2)
                    nc.sync.dma_start(
                        out=rhs_f, in_=rhs_d[kt * P:(kt + 1) * P, :])
                    for nb in range(NB):
                        psT = rhs_tp_psum.tile([P, P], f32, name="psT", tag="psT")
                        nc.tensor.transpose(
                            psT, rhs_f[:, nb * P:(nb + 1) * P], ident)
                        nc.any.tensor_copy(
                            out=rhsT[:, nb, kt * P:(kt + 1) * P], in_=psT)

                # per-column absmax over K (exact, f32), scales
                namax = rhs_misc.tile([P, NB], f32, name="namax")
                for nb in range(NB):
                    nc.vector.tensor_reduce(
                        out=namax[:, nb:nb + 1], in_=rhsT[:, nb, :],
                        axis=mybir.AxisListType.X, op=mybir.AluOpType.max,
                        apply_absolute_value=True,
                    )
                s_col = rhs_misc.tile([P, NB], f32, name="s_col")
                nc.vector.tensor_scalar_mul(s_col, namax, INV127)
                rs_col = rhs_misc.tile([P, NB], f32, name="rs_col")
                nc.vector.reciprocal(rs_col, s_col)

                # quantize+fold col-scale in transposed layout, xbar back
                for nb in range(NB):
                    yT = rhs_q_pool.tile([P, K], f32, name="yT", tag="yT")
                    nc.scalar.activation(
                        out=yT, in_=rhsT[:, nb, :],
                        func=mybir.ActivationFunctionType.Copy,
                        bias=MAGIC, scale=rs_col[:, nb:nb + 1],
                    )
                    qiT = rhs_q_pool.tile([P, K], f32, name="qiT", tag="qiT")
                    nc.vector.tensor_scalar(
                        out=qiT, in0=yT, scalar1=MAGIC, scalar2=None,
                        op0=mybir.AluOpType.subtract,
                    )
                    qsT = rhs_q_pool.tile([P, K], bf16, name="qsT", tag="qsT")
                    nc.scalar.activation(
                        out=qsT, in_=qiT,
                        func=mybir.ActivationFunctionType.Copy,
                        bias=0.0, scale=s_col[:, nb:nb + 1],
                    )
                    qtmp = rhs_q_pool.tile(
                        [P, KT, P], bf16, name=f"qtmp{nb}", tag="qtmp", bufs=NB)
                    nc.sync.dma_start(out=qtmp, in_=qsT, transpose=True)
                    # repack into q_rhs [k_part, kt, n] (moving operand layout)
                    nc.vector.tensor_copy(
                        out=q_rhs[:, :, nb * P:(nb + 1) * P], in_=qtmp)

            # ---------------- lhs main loop ------------------------------
            for g in range(ROW_TILES // CHUNK):
                xs = []
                amax = scales_pool.tile([P, CHUNK], f32, name="amax", tag="amax")
                for j in range(CHUNK):
                    i = g * CHUNK + j
                    x = lhsx_pool.tile([P, K], f32, name=f"x{j}", tag=f"x{j}")
                    nc.sync.dma_start(out=x, in_=lhs_d[i * P:(i + 1) * P, :])
                    nc.vector.tensor_reduce(
                        out=amax[:, j:j + 1], in_=x,
                        axis=mybir.AxisListType.X, op=mybir.AluOpType.max,
                        apply_absolute_value=True,
                    )
                    xs.append(x)
                s_row = scales_pool.tile([P, CHUNK], f32, name="s_row", tag="s_row")
                nc.vector.tensor_scalar_mul(s_row, amax, INV127)
                rs_row = scales_pool.tile([P, CHUNK], f32, name="rs_row", tag="rs_row")
                nc.vector.reciprocal(rs_row, s_row)

                for j in range(CHUNK):
                    i = g * CHUNK + j
                    x = xs[j]
                    y = y_pool.tile([P, K], f32, name="y", tag="y")
                    nc.scalar.activation(
                        out=y, in_=x,
                        func=mybir.ActivationFunctionType.Copy,
                        bias=MAGIC, scale=rs_row[:, j:j + 1],
                    )
                    q = q_pool.tile([P, K], bf16, name="q", tag="q")
                    nc.vector.tensor_scalar(
                        out=q, in0=y, scalar1=MAGIC, scalar2=None,
                        op0=mybir.AluOpType.subtract,
                    )
                    qlT = qlt_pool.tile([P, KT, P], bf16, name="qlT", tag="qlT")
                    nc.sync.dma_start(out=qlT, in_=q, transpose=True)

                    ps0 = mm_psum.tile([P, 512], f32, name="ps0", tag="ps0")
                    ps1 = mm_psum.tile([P, 512], f32, name="ps1", tag="ps1")
                    for b in range(KT):
                        nc.tensor.matmul(
                            ps0, qlT[:, b, :], q_rhs[:, b, 0:512],
                            start=(b == 0), stop=(b == KT - 1),
                        )
                    for b in range(KT):
                        nc.tensor.matmul(
                            ps1, qlT[:, b, :], q_rhs[:, b, 512:1024],
                            start=(b == 0), stop=(b == KT - 1),
                        )
                    o = o_pool.tile([P, N], f32, name="o", tag="o")
                    nc.scalar.activation(
                        out=o[:, 0:512], in_=ps0,
                        func=mybir.ActivationFunctionType.Copy,
                        bias=0.0, scale=s_row[:, j:j + 1],
                    )
                    nc.scalar.activation(
                        out=o[:, 512:1024], in_=ps1,
                        func=mybir.ActivationFunctionType.Copy,
                        bias=0.0, scale=s_row[:, j:j + 1],
                    )
                    nc.sync.dma_start(out=out_d[i * P:(i + 1) * P, :], in_=o)

    nc.compile()
    return nc


def _get_nc():
    if "nc" not in _CACHE:
        _CACHE["nc"] = _build()
    return _CACHE["nc"]


def kernel(lhs: np.ndarray, rhs: np.ndarray) -> np.ndarray:
    from concourse.bass_utils import run_bass_kernel_spmd

    nc = _get_nc()
    lhs_flat = np.ascontiguousarray(lhs.reshape(ROWS_TOTAL, K), dtype=np.float32)
    rhs = np.ascontiguousarray(rhs, dtype=np.float32)
    in_maps = [
        {
            "lhs": lhs_flat[c * ROWS_PER_CORE:(c + 1) * ROWS_PER_CORE],
            "rhs": rhs,
        }
        for c in range(N_CORES)
    ]
    res = run_bass_kernel_spmd(nc, in_maps, core_ids=list(range(N_CORES)))
    out = np.concatenate([res.results[c]["out"] for c in range(N_CORES)], axis=0)
    return out.reshape(B, S, N)
